# revision 6
# baseline (speedup 1.0000x reference)
"""Trainium2 Bass kernel for the GBM sampling-loss problem (v3).

Contract: kernel(**inputs) takes the FULL unsharded inputs
  x[2,500,3,128,128] z[2,3,128,128] Wm[6,3,3,3] bm[6] temb_w[6] t[2]
and returns the scalar loss (np.float32, shape ()).

v3 redesign vs the 3788ns v2, driven by trace analysis:
  - conv via a 10-partition layout [(c,dy)+ones, 32*132 shifted rows]:
    96 tiny matmuls (3-way PSUM accumulation over dx) replace the
    112-partition im2col, shrinking the patch DMA 500ns -> 236ns and
    letting it land first on SP while win rides the Pool SWDGE queue
    (both transfers overlap; each DMA occupies its issuing engine for
    the duration of its transfer).
  - ACT carries only Exp and ONE Ln: ln(var_ratio) = ln(49*sig^2/v6)
    + const, so u = 49*sig^2/v6 (one DVE STT divide) replaces the
    separate lnS/lnG pair; Relu-clamp moves to Pool (TT max vs an EPS
    tile read straight from PSUM) and the p_mu PSUM copy is gone (Pool
    reads the 2*p_mu channels from PSUM directly).
  - KL t1 term via (3.5*(2pm) - musum)^2 / v6 with STT accumulate.
  - Output: 4 per-partition column sums -> one partition_all_reduce ->
    Pool register stores (no output DMA).
  - Every cross-engine consumer of a DMA is scheduled to issue after
    the transfer-end (filler memsets / engine busyness), so no consumer
    pays the +1717ns DMA-semaphore path.

Engine layout:
  SP   : mpat DMA, wz DMA
  Pool : win DMA (SWDGE), musum tree, bt2, clamp, a1/t2/ein chain,
         sg2/dm/dmu/d7sq, xt/d, partition_all_reduce + register stores
  DVE  : const memsets (double as late-wait fillers), sq tree, v6,
         u/c2 STT divides (with accum cols), |d| reduce
  ACT  : [ATL], Exp, Ln(u) with accum col
  PE   : 96 tiny conv matmuls (dx-accumulated), gated on a DVE sem
"""

import os
import sys

sys.path.insert(0, "/opt/trn_rl_repo")

import numpy as np

try:
    import ml_dtypes
    NPBF16 = np.dtype(ml_dtypes.bfloat16)
except ImportError:  # pragma: no cover
    import jax.numpy as jnp
    NPBF16 = np.dtype(jnp.bfloat16)

K = 3
T = 500
C = 3
B = 2
H = 128
W = 128
EPS = 1e-7
N_CORES = 8
HS = H // 4          # 32 rows per core
N_TOT = B * C * H * W  # 98304 elements in the loss means
MCOLS = HS * 132     # 4224 im2col-lite columns
MW = MCOLS + 18      # + 3 dx-blocks of 6 weight columns
LN67 = float(np.log(6.0 / 7.0))

_built = None
LAST_RESULTS = None


def _build_nc():
    import concourse.bacc as bacc
    import concourse.mybir as mybir
    from concourse import bass_isa
    from concourse import tile as tile_mod

    f32 = mybir.dt.float32
    f16 = mybir.dt.bfloat16
    i32 = mybir.dt.int32
    AF = mybir.ActivationFunctionType
    ALU = mybir.AluOpType
    AX = mybir.AxisListType
    ET = mybir.EngineType

    nc = bacc.Bacc()

    mpat_d = nc.dram_tensor("mpat", [10, MW], f16, kind="ExternalInput")
    win_d = nc.dram_tensor("win", [128, 7 * HS * C], f16, kind="ExternalInput")
    wz_d = nc.dram_tensor("wz", [128, HS * C], f16, kind="ExternalInput")
    out_d = nc.dram_tensor("out", [1, 4], f32, kind="ExternalOutput")

    with tile_mod.TileContext(nc) as tc:
        with (
            tc.tile_pool(name="sb", bufs=1) as sb,
            tc.tile_pool(name="ps", bufs=1, space="PSUM") as ps,
        ):
            mpat = sb.tile([10, MW], f16)
            win = sb.tile([128, 7, HS, C], f16)
            wz = sb.tile([128, HS, C], f16)

            # Pool carries the big win DMA on the SWDGE queue (the Pool
            # engine is held for the whole transfer, which conveniently
            # makes its first consumer register its wait late).
            nc.gpsimd.dma_start(out=win[:].rearrange("p s h c -> p (s h c)"), in_=win_d[:])
            i_mdma = nc.sync.dma_start(out=mpat[:], in_=mpat_d[:])
            gate = nc.alloc_semaphore("pe_gate")
            i_ginc = nc.sync.sem_inc(gate, 1)
            tile_mod.add_dep_helper(i_ginc.ins, i_mdma.ins, reason="gate after mpat")
            i_wzdma = nc.sync.dma_start(out=wz[:].rearrange("p h c -> p (h c)"), in_=wz_d[:])
            tile_mod.add_dep_helper(i_wzdma.ins, i_mdma.ins, reason="SP dma order")

            # Explicit ACT table load (covers Exp + Ln); runs during the
            # DMAs and finishes at ~1483.
            tabs = bacc.get_activation_tables(nc.m.arch)
            set_id = list(tabs).index("natural_log_exp_and_others")
            atl = mybir.InstLoadActFuncSet(
                name=nc.get_next_instruction_name(), ins=[], outs=[],
                act_func_set_id=set_id,
            )
            i_atl = nc.scalar.add_instruction(atl)

            # DVE const tiles double as late-wait fillers: they keep DVE
            # busy past the mpat (436) and win (~580) transfer-ends so
            # the PE gate fires late and sq's wait registers late.
            ceps = sb.tile([128, HS, C], f32)
            nc.vector.memset(ceps[:], EPS)
            c35 = sb.tile([128, HS, C], f32)
            i_c35 = nc.vector.memset(c35[:], 3.5)
            sq = sb.tile([128, 8, HS, C], f16)
            i_ms7 = nc.vector.memset(sq[:, 7:8], 0.0)
            tile_mod.add_dep_helper(i_ms7.ins, i_c35.ins, reason="DVE order")

            # ---- conv: err[x, y, o] via 96 accumulating matmuls ----
            # out channels 0:3 = sigma', 3:6 = 2*p_mu (bias+temb folded
            # into the ones-partition rhs row of the dx=0 block).
            i_gatew = nc.tensor.wait_ge(gate, 1)
            err_ps = ps.tile([128, HS, 6], f32)
            first_mm = None
            for y in range(HS):
                for dx in range(3):
                    i_mm = nc.tensor.matmul(
                        err_ps[:, y, :],
                        mpat[0:10, y * 132 + dx : y * 132 + dx + 128],
                        mpat[0:10, MCOLS + 6 * dx : MCOLS + 6 * dx + 6],
                        start=(dx == 0),
                        stop=(dx == 2),
                    )
                    if first_mm is None:
                        first_mm = i_mm
                        tile_mod.add_dep_helper(i_mm.ins, i_gatew.ins, reason="PE gate")

            sig_ps = err_ps[:, :, 0:3]
            pm2_ps = err_ps[:, :, 3:6]

            # ---- Pool: musum tree, then the sampling/KL TT chain ----
            def pool_tt(name_tile, a, b, op):
                t = sb.tile([128, HS, C], f32, name=name_tile)
                ins = nc.gpsimd.tensor_tensor(t[:], a, b, op=op)
                return t, ins

            m01, i1 = pool_tt("m01", win[:, 0], win[:, 1], ALU.add)
            m23, i2 = pool_tt("m23", win[:, 2], win[:, 3], ALU.add)
            tile_mod.add_dep_helper(i2.ins, i1.ins, reason="pool order")
            m45, i3 = pool_tt("m45", win[:, 4], win[:, 5], ALU.add)
            tile_mod.add_dep_helper(i3.ins, i2.ins, reason="pool order")
            mA, i4 = pool_tt("mA", m01[:], m23[:], ALU.add)
            tile_mod.add_dep_helper(i4.ins, i3.ins, reason="pool order")
            mB, i5 = pool_tt("mB", m45[:], win[:, 6], ALU.add)
            tile_mod.add_dep_helper(i5.ins, i4.ins, reason="pool order")
            musum, i6 = pool_tt("musum", mA[:], mB[:], ALU.add)
            tile_mod.add_dep_helper(i6.ins, i5.ins, reason="pool order")
            bt2, i7 = pool_tt("bt2", musum[:], musum[:], ALU.mult)
            tile_mod.add_dep_helper(i7.ins, i6.ins, reason="pool order")
            sgs, i8 = pool_tt("sgs", sig_ps, ceps[:], ALU.max)
            tile_mod.add_dep_helper(i8.ins, i7.ins, reason="pool order")
            a1, i9 = pool_tt("a1", sgs[:], wz[:], ALU.add)
            tile_mod.add_dep_helper(i9.ins, i8.ins, reason="pool order")
            t2, i10 = pool_tt("t2", sgs[:], a1[:], ALU.mult)
            tile_mod.add_dep_helper(i10.ins, i9.ins, reason="pool order")
            ein, i11 = pool_tt("ein", t2[:], pm2_ps, ALU.add)
            tile_mod.add_dep_helper(i11.ins, i10.ins, reason="pool order")
            sg2, i12 = pool_tt("sg2", sgs[:], sgs[:], ALU.mult)
            tile_mod.add_dep_helper(i12.ins, i11.ins, reason="pool order")
            dm, i13 = pool_tt("dm", pm2_ps, c35[:], ALU.mult)
            tile_mod.add_dep_helper(i13.ins, i12.ins, reason="pool order")
            dmu, i14 = pool_tt("dmu", dm[:], musum[:], ALU.subtract)
            tile_mod.add_dep_helper(i14.ins, i13.ins, reason="pool order")
            d7sq, i15 = pool_tt("d7sq", dmu[:], dmu[:], ALU.mult)
            tile_mod.add_dep_helper(i15.ins, i14.ins, reason="pool order")

            # ---- DVE: squares tree -> ssq -> v6 -> u,c2 divides ----
            nc.vector.tensor_tensor(sq[:, 0:7], win[:], win[:], op=ALU.mult)
            u4 = sb.tile([128, 4, HS, C], f16)
            nc.vector.tensor_tensor(u4[:], sq[:, 0:4], sq[:, 4:8], op=ALU.add)
            u2 = sb.tile([128, 2, HS, C], f16)
            nc.vector.tensor_tensor(u2[:], u4[:, 0:2], u4[:, 2:4], op=ALU.add)
            ssq = sb.tile([128, HS, C], f32)
            nc.vector.tensor_tensor(ssq[:], u2[:, 0], u2[:, 1], op=ALU.add)
            v6 = sb.tile([128, HS, C], f32)
            nc.vector.scalar_tensor_tensor(
                v6[:], ssq[:], 7.0, bt2[:], op0=ALU.mult, op1=ALU.subtract
            )
            cols = sb.tile([128, 4], f32)
            u = sb.tile([128, HS, C], f32)
            i_u = nc.vector.scalar_tensor_tensor(
                u[:], sg2[:], 49.0, v6[:], op0=ALU.mult, op1=ALU.divide,
                accum_out=cols[:, 1:2],
            )
            c2 = sb.tile([128, HS, C], f32)
            i_c2 = nc.vector.scalar_tensor_tensor(
                c2[:], d7sq[:], 1.0, v6[:], op0=ALU.mult, op1=ALU.divide,
                accum_out=cols[:, 2:3],
            )
            tile_mod.add_dep_helper(i_c2.ins, i_u.ins, reason="DVE u first")

            # ---- ACT: Exp then Ln(u), both after the ATL ----
            e = sb.tile([128, HS, C], f32)
            i_e = nc.scalar.activation(e[:], ein[:], AF.Exp, scale=0.5)
            tile_mod.add_dep_helper(i_e.ins, i_atl.ins, reason="ACT order atl<e")
            lnu = sb.tile([128, HS, C], f32)
            i_lnu = nc.scalar.activation(
                lnu[:], u[:], AF.Ln, accum_out=cols[:, 3:4]
            )
            tile_mod.add_dep_helper(i_lnu.ins, i_e.ins, reason="ACT order e<lnu")

            # ---- sampling tail ----
            xt, i16 = pool_tt("xt", e[:], win[:, 2], ALU.mult)
            tile_mod.add_dep_helper(i16.ins, i15.ins, reason="pool order")
            d, i17 = pool_tt("d", xt[:], win[:, 3], ALU.subtract)
            tile_mod.add_dep_helper(i17.ins, i16.ins, reason="pool order")
            i_dred = nc.vector.tensor_reduce(
                cols[:, 0:1], d[:], axis=AX.XY, op=ALU.add,
                apply_absolute_value=True,
            )
            tile_mod.add_dep_helper(i_dred.ins, i_c2.ins, reason="DVE order")

            # ---- final: one partition all-reduce + register stores ----
            red = sb.tile([128, 4], f32)
            i_par = nc.gpsimd.partition_all_reduce(
                red[:], cols[:], 128, bass_isa.ReduceOp.add
            )
            tile_mod.add_dep_helper(i_par.ins, i17.ins, reason="pool order")
            regs = [nc.alloc_register(ET.Pool, f"acc{i}") for i in range(4)]
            nc.gpsimd.reg_load(regs, red[0:1, 0:4].bitcast(i32))
            for i in range(4):
                nc.gpsimd.store(out=out_d[0:1, i : i + 1].bitcast(i32), in_=regs[i])

    # The explicit ATL above covers Exp and Ln; suppress the
    # compile-time auto-inserter (it would add a second 1283ns load).
    nc.insert_act_table_loads = lambda: None
    nc.compile()
    return nc


def _prep_inputs(x, z, Wm, bm, temb_w, t):
    """Build the 8 per-core input dicts (pure numpy, host side)."""
    x = np.ascontiguousarray(np.asarray(x, dtype=np.float32))
    z = np.asarray(z, dtype=np.float32)
    Wm = np.asarray(Wm, dtype=np.float32)
    bm = np.asarray(bm, dtype=np.float32)
    temb_w = np.asarray(temb_w, dtype=np.float32)
    t = np.asarray(t)

    in_maps = []
    for b in range(B):
        ti = int(t[b])
        st = min(max(ti - K, 0), T - (2 * K + 1))
        window = x[b, st : st + 2 * K + 1]          # [7,3,128,128]
        xin = window[K - 1]                          # [3,128,128]
        bias = bm + temb_w * (np.float32(ti) / np.float32(T))
        sqt2 = np.float32(2.0 * np.sqrt(np.float64(ti)))

        # weight columns, shared across quarters
        wcols = np.zeros((3, 10, 6), np.float32)     # [dx, p, o]
        for dx in range(3):
            for c in range(C):
                for dy in range(3):
                    p = c * 3 + dy
                    wcols[dx, p, 0:3] = Wm[3:6, c, dy, dx]
                    wcols[dx, p, 3:6] = 2.0 * Wm[0:3, c, dy, dx]
        wcols[0, 9, 0:3] = bias[3:6]
        wcols[0, 9, 3:6] = 2.0 * bias[0:3]

        for q in range(4):
            r0 = q * HS
            # win: [w, s, h, c]
            wina = window[:, :, r0 : r0 + HS, :].transpose(3, 0, 2, 1)
            wina = np.ascontiguousarray(wina).reshape(128, 7 * HS * C).astype(NPBF16)
            # wz: [w, h, c]
            wzv = (sqt2 * z[b, :, r0 : r0 + HS, :]).transpose(2, 1, 0)
            wzv = np.ascontiguousarray(wzv).reshape(128, HS * C).astype(NPBF16)
            # mpat: padded shifted rows per (c, dy) + ones + weights
            XP = np.zeros((C, HS + 2, 132), np.float32)
            XP[:, 1 : HS + 1, 1:129] = xin[:, r0 : r0 + HS, :]
            if r0 > 0:
                XP[:, 0, 1:129] = xin[:, r0 - 1, :]
            if r0 + HS < H:
                XP[:, HS + 1, 1:129] = xin[:, r0 + HS, :]
            mpat = np.zeros((10, MW), np.float32)
            for c in range(C):
                for dy in range(3):
                    mpat[c * 3 + dy, :MCOLS] = XP[c, dy : dy + HS, :].reshape(-1)
            mpat[9, :MCOLS] = 1.0
            for dx in range(3):
                mpat[:, MCOLS + 6 * dx : MCOLS + 6 * dx + 6] = wcols[dx]
            in_maps.append({
                "mpat": mpat.astype(NPBF16),
                "win": wina,
                "wz": wzv,
            })
    return in_maps


def _combine(results):
    outs = np.stack([np.asarray(r["out"], dtype=np.float64) for r in results])
    s = outs.sum(axis=0)[0]  # [4]: sum|d|, sum u, sum d7sq/v6, sum ln u
    l1 = s[0] / N_TOT
    kl_sum = 0.5 * ((6.0 / 7.0) * (s[1] + s[2]) - s[3] - N_TOT * LN67 - N_TOT)
    return np.float32(l1 + kl_sum / N_TOT)


def kernel(x, z, Wm, bm, temb_w, t):
    global _built, LAST_RESULTS
    from concourse.bass_utils import run_bass_kernel_spmd

    if _built is None:
        _built = _build_nc()
    nc = _built

    in_maps = _prep_inputs(x, z, Wm, bm, temb_w, t)
    trace = bool(os.environ.get("BASS_TRACE"))
    res = run_bass_kernel_spmd(nc, in_maps, core_ids=list(range(N_CORES)), trace=trace)
    LAST_RESULTS = res
    return _combine(res.results)


# revision 10
# speedup vs baseline: 1.3404x; 1.3404x over previous
"""Trainium2 Bass kernel for the GBM sampling-loss problem (v3).

Contract: kernel(**inputs) takes the FULL unsharded inputs
  x[2,500,3,128,128] z[2,3,128,128] Wm[6,3,3,3] bm[6] temb_w[6] t[2]
and returns the scalar loss (np.float32, shape ()).

v3 redesign vs the 3788ns v2, driven by trace analysis:
  - conv via an 8-group x 10-partition layout [(g,c,dy)+ones, 4*132
    shifted row cols + weights]: 1092B/partition hits the ~500ns DMA
    floor (DMA cost is per-partition bytes, not total), vs 500ns for
    the v2 im2col pair. 2x96 tiny matmuls with 3-way PSUM accumulation
    over dx; the sigma channels run as a separate first pass so the
    Pool clamp chain starts ~250ns before the pm2 channels finish.
  - ACT carries only Exp and ONE Ln: ln(var_ratio) = ln(49*sig^2/v6)
    + const, so u = 49*sig^2/v6 (one DVE STT divide) replaces the
    v2 lnS/lnG pair; the Relu clamp moves to Pool (TT max vs an EPS
    tile, reading sigma straight from PSUM) and the p_mu PSUM copy is
    gone (Pool reads the 2*p_mu channels from PSUM directly).
  - KL t1 term via (3.5*(2pm) - musum)^2 / v6 with STT accumulate.
  - wz rides the win DMA as slab 7 (one [128, 8, 32, 3] SWDGE DMA).
  - Output: 4 per-partition column sums -> one partition_all_reduce ->
    Pool register stores (no output DMA).
  - Every cross-engine consumer of a DMA is scheduled to issue after
    the transfer-end (filler memsets / engine busyness), so no consumer
    pays the +1717ns DMA-semaphore path.

Engine layout:
  SP   : mpat DMA, then the PE gate sem
  Pool : win DMA (SWDGE), musum tree, bt2, clamp, a1/t2/ein chain,
         sg2/dm/dmu/d7sq, xt/d, partition_all_reduce + register stores
  DVE  : const memsets (double as late-wait fillers), sq tree, v6,
         u/c2 STT divides (with accum cols), |d| reduce
  ACT  : [ATL], Exp, Ln(u) with accum col
  PE   : 2x96 tiny conv matmuls (dx-accumulated), gated on the SP sem
"""

import os
import sys

sys.path.insert(0, "/opt/trn_rl_repo")

import numpy as np

try:
    import ml_dtypes
    NPBF16 = np.dtype(ml_dtypes.bfloat16)
except ImportError:  # pragma: no cover
    import jax.numpy as jnp
    NPBF16 = np.dtype(jnp.bfloat16)

K = 3
T = 500
C = 3
B = 2
H = 128
W = 128
EPS = 1e-7
N_CORES = 8
HS = H // 4            # 32 rows per core
N_TOT = B * C * H * W  # 98304 elements in the loss means
NG = 8                 # y-groups per core
GR = HS // NG          # 4 rows per group
GCOLS = GR * 132       # 528 shifted-row cols per group
GW = GCOLS + NG * 18   # + per-group masked weight blocks (g,dx,half)
LN67 = float(np.log(6.0 / 7.0))

_built = None
LAST_RESULTS = None


def _build_nc():
    import concourse.bacc as bacc
    import concourse.mybir as mybir
    from concourse import bass_isa
    from concourse import tile as tile_mod

    f32 = mybir.dt.float32
    f16 = mybir.dt.bfloat16
    i32 = mybir.dt.int32
    AF = mybir.ActivationFunctionType
    ALU = mybir.AluOpType
    AX = mybir.AxisListType
    ET = mybir.EngineType

    nc = bacc.Bacc()

    mpat_d = nc.dram_tensor("mpat", [NG * 10, GW], f16, kind="ExternalInput")
    win_d = nc.dram_tensor("win", [128, 8 * HS * C], f16, kind="ExternalInput")
    out_d = nc.dram_tensor("out", [1, 4], f32, kind="ExternalOutput")

    with tile_mod.TileContext(nc) as tc:
        with (
            tc.tile_pool(name="sb", bufs=1) as sb,
            tc.tile_pool(name="ps", bufs=1, space="PSUM") as ps,
        ):
            mpat = sb.tile([NG * 10, GW], f16)
            win = sb.tile([128, 8, HS, C], f16)

            # Pool carries the big win(+wz) DMA on the SWDGE queue; the
            # Pool engine is held for the whole transfer, which makes
            # its first consumer (m01) register its wait late for free.
            nc.gpsimd.dma_start(out=win[:].rearrange("p s h c -> p (s h c)"), in_=win_d[:])
            i_mdma = nc.sync.dma_start(out=mpat[:], in_=mpat_d[:])
            gate = nc.alloc_semaphore("pe_gate")
            i_ginc = nc.sync.sem_inc(gate, 1)
            tile_mod.add_dep_helper(i_ginc.ins, i_mdma.ins, reason="gate after mpat")

            # Explicit ACT table load (covers Exp + Ln); runs during the
            # DMAs and finishes at ~1483.
            tabs = bacc.get_activation_tables(nc.m.arch)
            set_id = list(tabs).index("natural_log_exp_and_others")
            atl = mybir.InstLoadActFuncSet(
                name=nc.get_next_instruction_name(), ins=[], outs=[],
                act_func_set_id=set_id,
            )
            i_atl = nc.scalar.add_instruction(atl)

            # DVE const tiles double as late-wait fillers: they keep DVE
            # busy past the win transfer-end (~690) so sq's wait
            # registers late.
            ceps = sb.tile([128, HS, C], f32)
            nc.vector.memset(ceps[:], EPS)
            c35 = sb.tile([128, HS, C], f32)
            i_c35 = nc.vector.memset(c35[:], 3.5)
            sq = sb.tile([128, 8, HS, C], f16)
            i_ms7 = nc.vector.memset(sq[:, 7:8], 0.0)
            tile_mod.add_dep_helper(i_ms7.ins, i_c35.ins, reason="DVE order")
            fill = sb.tile([128, HS, C], f16)
            i_f4 = nc.vector.memset(fill[:], 0.0)
            tile_mod.add_dep_helper(i_f4.ins, i_ms7.ins, reason="DVE order")

            # ---- conv: err[x, y, o] via accumulating matmuls ----
            # sigma channels first (unblocks the Pool clamp chain), then
            # the 2*p_mu channels. bias+temb folded into the ones-row of
            # the dx=0 weight block.
            # The contraction spans all 80 partitions from base 0 (PE
            # requires base 0/32/64); per-group selection happens via
            # zero-masked weight column blocks.
            i_gatew = nc.tensor.wait_ge(gate, 1)
            sig_ps = ps.tile([128, HS, C], f32)
            pm2_ps = ps.tile([128, HS, C], f32)
            first_mm = None
            for half in range(2):
                for y in range(HS):
                    g, yy = divmod(y, GR)
                    out_ps = sig_ps if half == 0 else pm2_ps
                    for dx in range(3):
                        wc = GCOLS + (g * 6 + dx * 2 + half) * 3
                        i_mm = nc.tensor.matmul(
                            out_ps[:, y, :],
                            mpat[0 : NG * 10,
                                 yy * 132 + dx : yy * 132 + dx + 128],
                            mpat[0 : NG * 10, wc : wc + 3],
                            start=(dx == 0),
                            stop=(dx == 2),
                        )
                        if first_mm is None:
                            first_mm = i_mm
                            tile_mod.add_dep_helper(i_mm.ins, i_gatew.ins, reason="PE gate")

            # ---- Pool: musum tree, then the sampling/KL TT chain ----
            def pool_tt(name_tile, a, b, op):
                t = sb.tile([128, HS, C], f32, name=name_tile)
                ins = nc.gpsimd.tensor_tensor(t[:], a, b, op=op)
                return t, ins

            m01, i1 = pool_tt("m01", win[:, 0], win[:, 1], ALU.add)
            m23, i2 = pool_tt("m23", win[:, 2], win[:, 3], ALU.add)
            tile_mod.add_dep_helper(i2.ins, i1.ins, reason="pool order")
            m45, i3 = pool_tt("m45", win[:, 4], win[:, 5], ALU.add)
            tile_mod.add_dep_helper(i3.ins, i2.ins, reason="pool order")
            mA, i4 = pool_tt("mA", m01[:], m23[:], ALU.add)
            tile_mod.add_dep_helper(i4.ins, i3.ins, reason="pool order")
            mB, i5 = pool_tt("mB", m45[:], win[:, 6], ALU.add)
            tile_mod.add_dep_helper(i5.ins, i4.ins, reason="pool order")
            musum, i6 = pool_tt("musum", mA[:], mB[:], ALU.add)
            tile_mod.add_dep_helper(i6.ins, i5.ins, reason="pool order")
            bt2, i7 = pool_tt("bt2", musum[:], musum[:], ALU.mult)
            tile_mod.add_dep_helper(i7.ins, i6.ins, reason="pool order")
            sgs, i8 = pool_tt("sgs", sig_ps[:], ceps[:], ALU.max)
            tile_mod.add_dep_helper(i8.ins, i7.ins, reason="pool order")
            a1, i9 = pool_tt("a1", sgs[:], win[:, 7], ALU.add)
            tile_mod.add_dep_helper(i9.ins, i8.ins, reason="pool order")
            t2, i10 = pool_tt("t2", sgs[:], a1[:], ALU.mult)
            tile_mod.add_dep_helper(i10.ins, i9.ins, reason="pool order")
            ein, i11 = pool_tt("ein", t2[:], pm2_ps[:], ALU.add)
            tile_mod.add_dep_helper(i11.ins, i10.ins, reason="pool order")
            sg2, i12 = pool_tt("sg2", sgs[:], sgs[:], ALU.mult)
            tile_mod.add_dep_helper(i12.ins, i11.ins, reason="pool order")
            dm, i13 = pool_tt("dm", pm2_ps[:], c35[:], ALU.mult)
            tile_mod.add_dep_helper(i13.ins, i12.ins, reason="pool order")
            dmu, i14 = pool_tt("dmu", dm[:], musum[:], ALU.subtract)
            tile_mod.add_dep_helper(i14.ins, i13.ins, reason="pool order")
            d7sq, i15 = pool_tt("d7sq", dmu[:], dmu[:], ALU.mult)
            tile_mod.add_dep_helper(i15.ins, i14.ins, reason="pool order")

            # ---- DVE: squares tree -> ssq -> v6 -> u,c2 divides ----
            nc.vector.tensor_tensor(sq[:, 0:7], win[:, 0:7], win[:, 0:7], op=ALU.mult)
            u4 = sb.tile([128, 4, HS, C], f16)
            nc.vector.tensor_tensor(u4[:], sq[:, 0:4], sq[:, 4:8], op=ALU.add)
            u2 = sb.tile([128, 2, HS, C], f16)
            nc.vector.tensor_tensor(u2[:], u4[:, 0:2], u4[:, 2:4], op=ALU.add)
            ssq = sb.tile([128, HS, C], f32)
            nc.vector.tensor_tensor(ssq[:], u2[:, 0], u2[:, 1], op=ALU.add)
            v6 = sb.tile([128, HS, C], f32)
            nc.vector.scalar_tensor_tensor(
                v6[:], ssq[:], 7.0, bt2[:], op0=ALU.mult, op1=ALU.subtract
            )
            cols = sb.tile([128, 4], f32)
            u = sb.tile([128, HS, C], f32)
            i_u = nc.vector.scalar_tensor_tensor(
                u[:], sg2[:], 49.0, v6[:], op0=ALU.mult, op1=ALU.divide,
                accum_out=cols[:, 1:2],
            )
            c2 = sb.tile([128, HS, C], f32)
            i_c2 = nc.vector.scalar_tensor_tensor(
                c2[:], d7sq[:], 1.0, v6[:], op0=ALU.mult, op1=ALU.divide,
                accum_out=cols[:, 2:3],
            )
            tile_mod.add_dep_helper(i_c2.ins, i_u.ins, reason="DVE u first")

            # ---- ACT: Exp then Ln(u), both after the ATL ----
            e = sb.tile([128, HS, C], f32)
            i_e = nc.scalar.activation(e[:], ein[:], AF.Exp, scale=0.5)
            tile_mod.add_dep_helper(i_e.ins, i_atl.ins, reason="ACT order atl<e")
            lnu = sb.tile([128, HS, C], f32)
            i_lnu = nc.scalar.activation(
                lnu[:], u[:], AF.Ln, accum_out=cols[:, 3:4]
            )
            tile_mod.add_dep_helper(i_lnu.ins, i_e.ins, reason="ACT order e<lnu")

            # ---- sampling tail ----
            xt, i16 = pool_tt("xt", e[:], win[:, 2], ALU.mult)
            tile_mod.add_dep_helper(i16.ins, i15.ins, reason="pool order")
            d, i17 = pool_tt("d", xt[:], win[:, 3], ALU.subtract)
            tile_mod.add_dep_helper(i17.ins, i16.ins, reason="pool order")
            i_dred = nc.vector.tensor_reduce(
                cols[:, 0:1], d[:], axis=AX.XY, op=ALU.add,
                apply_absolute_value=True,
            )
            tile_mod.add_dep_helper(i_dred.ins, i_c2.ins, reason="DVE order")

            # ---- final: one partition all-reduce + register stores ----
            red = sb.tile([128, 4], f32)
            i_par = nc.gpsimd.partition_all_reduce(
                red[:], cols[:], 128, bass_isa.ReduceOp.add
            )
            tile_mod.add_dep_helper(i_par.ins, i17.ins, reason="pool order")
            regs = [nc.alloc_register(ET.Pool, f"acc{i}") for i in range(4)]
            nc.gpsimd.reg_load(regs, red[0:1, 0:4].bitcast(i32))
            for i in range(4):
                nc.gpsimd.store(out=out_d[0:1, i : i + 1].bitcast(i32), in_=regs[i])

    # The explicit ATL above covers Exp and Ln; suppress the
    # compile-time auto-inserter (it would add a second 1283ns load).
    nc.insert_act_table_loads = lambda: None
    nc.compile()
    return nc


def _prep_inputs(x, z, Wm, bm, temb_w, t):
    """Build the 8 per-core input dicts (pure numpy, host side)."""
    x = np.ascontiguousarray(np.asarray(x, dtype=np.float32))
    z = np.asarray(z, dtype=np.float32)
    Wm = np.asarray(Wm, dtype=np.float32)
    bm = np.asarray(bm, dtype=np.float32)
    temb_w = np.asarray(temb_w, dtype=np.float32)
    t = np.asarray(t)

    in_maps = []
    for b in range(B):
        ti = int(t[b])
        st = min(max(ti - K, 0), T - (2 * K + 1))
        window = x[b, st : st + 2 * K + 1]          # [7,3,128,128]
        xin = window[K - 1]                          # [3,128,128]
        bias = bm + temb_w * (np.float32(ti) / np.float32(T))
        sqt2 = np.float32(2.0 * np.sqrt(np.float64(ti)))

        # weight columns, shared across quarters: [dx, p(c,dy)+ones, o]
        wcols = np.zeros((3, 10, 6), np.float32)
        for dx in range(3):
            for c in range(C):
                for dy in range(3):
                    p = c * 3 + dy
                    wcols[dx, p, 0:3] = Wm[3:6, c, dy, dx]
                    wcols[dx, p, 3:6] = 2.0 * Wm[0:3, c, dy, dx]
        wcols[0, 9, 0:3] = bias[3:6]
        wcols[0, 9, 3:6] = 2.0 * bias[0:3]

        for q in range(4):
            r0 = q * HS
            # win: [w, s, h, c] with wz as slab 7
            wina = np.empty((128, 8, HS, C), np.float32)
            wina[:, 0:7] = window[:, :, r0 : r0 + HS, :].transpose(3, 0, 2, 1)
            wina[:, 7] = (sqt2 * z[b, :, r0 : r0 + HS, :]).transpose(2, 1, 0)
            wina = wina.reshape(128, 8 * HS * C).astype(NPBF16)
            # mpat: 8 groups of 4 padded shifted rows + ones + weights
            XP = np.zeros((C, HS + 2, 132), np.float32)
            XP[:, 1 : HS + 1, 1:129] = xin[:, r0 : r0 + HS, :]
            if r0 > 0:
                XP[:, 0, 1:129] = xin[:, r0 - 1, :]
            if r0 + HS < H:
                XP[:, HS + 1, 1:129] = xin[:, r0 + HS, :]
            mpat = np.zeros((NG * 10, GW), np.float32)
            for g in range(NG):
                y0 = g * GR
                for c in range(C):
                    for dy in range(3):
                        mpat[10 * g + c * 3 + dy, :GCOLS] = (
                            XP[c, y0 + dy : y0 + dy + GR, :].reshape(-1)
                        )
                mpat[10 * g + 9, :GCOLS] = 1.0
                for dx in range(3):
                    for half in range(2):
                        wc = GCOLS + (g * 6 + dx * 2 + half) * 3
                        mpat[10 * g : 10 * g + 10, wc : wc + 3] = (
                            wcols[dx, :, 3 * half : 3 * half + 3]
                        )
            in_maps.append({
                "mpat": mpat.astype(NPBF16),
                "win": wina,
            })
    return in_maps


def _combine(results):
    outs = np.stack([np.asarray(r["out"], dtype=np.float64) for r in results])
    s = outs.sum(axis=0)[0]  # [4]: sum|d|, sum u, sum d7sq/v6, sum ln u
    l1 = s[0] / N_TOT
    kl_sum = 0.5 * ((6.0 / 7.0) * (s[1] + s[2]) - s[3] - N_TOT * LN67 - N_TOT)
    return np.float32(l1 + kl_sum / N_TOT)


def kernel(x, z, Wm, bm, temb_w, t):
    global _built, LAST_RESULTS
    from concourse.bass_utils import run_bass_kernel_spmd

    if _built is None:
        _built = _build_nc()
    nc = _built

    in_maps = _prep_inputs(x, z, Wm, bm, temb_w, t)
    trace = bool(os.environ.get("BASS_TRACE"))
    res = run_bass_kernel_spmd(nc, in_maps, core_ids=list(range(N_CORES)), trace=trace)
    LAST_RESULTS = res
    return _combine(res.results)


# revision 12
# speedup vs baseline: 1.7442x; 1.3012x over previous
"""Trainium2 Bass kernel for the GBM sampling-loss problem (v3).

Contract: kernel(**inputs) takes the FULL unsharded inputs
  x[2,500,3,128,128] z[2,3,128,128] Wm[6,3,3,3] bm[6] temb_w[6] t[2]
and returns the scalar loss (np.float32, shape ()).

v3 redesign vs the 3788ns v2, driven by trace analysis:
  - conv via an 8-group x 10-partition layout [(g,c,dy)+ones, 4*132
    shifted row cols + weights]: 1092B/partition hits the ~500ns DMA
    floor (DMA cost is per-partition bytes, not total), vs 500ns for
    the v2 im2col pair. 2x96 tiny matmuls with 3-way PSUM accumulation
    over dx; the sigma channels run as a separate first pass so the
    Pool clamp chain starts ~250ns before the pm2 channels finish.
  - ACT carries only Exp and ONE Ln: ln(var_ratio) = ln(49*sig^2/v6)
    + const, so u = 49*sig^2/v6 (one DVE STT divide) replaces the
    v2 lnS/lnG pair; the Relu clamp moves to Pool (TT max vs an EPS
    tile, reading sigma straight from PSUM) and the p_mu PSUM copy is
    gone (Pool reads the 2*p_mu channels from PSUM directly).
  - KL t1 term via (3.5*(2pm) - musum)^2 / v6 with STT accumulate.
  - wz rides the win DMA as slab 7 (one [128, 8, 32, 3] SWDGE DMA).
  - Output: 4 per-partition column sums -> one partition_all_reduce ->
    Pool register stores (no output DMA).
  - Every cross-engine consumer of a DMA is scheduled to issue after
    the transfer-end (filler memsets / engine busyness), so no consumer
    pays the +1717ns DMA-semaphore path.

Engine layout:
  SP   : mpat DMA, then the PE gate sem
  Pool : win DMA (SWDGE), musum tree, bt2, clamp, a1/t2/ein chain,
         sg2/dm/dmu/d7sq, xt/d, partition_all_reduce + register stores
  DVE  : const memsets (double as late-wait fillers), sq tree, v6,
         u/c2 STT divides (with accum cols), |d| reduce
  ACT  : [ATL], Exp, Ln(u) with accum col
  PE   : 2x96 tiny conv matmuls (dx-accumulated), gated on the SP sem
"""

import os
import sys

sys.path.insert(0, "/opt/trn_rl_repo")

import numpy as np

try:
    import ml_dtypes
    NPBF16 = np.dtype(ml_dtypes.bfloat16)
except ImportError:  # pragma: no cover
    import jax.numpy as jnp
    NPBF16 = np.dtype(jnp.bfloat16)

K = 3
T = 500
C = 3
B = 2
H = 128
W = 128
EPS = 1e-7
N_CORES = 8
HS = H // 4            # 32 rows per core
N_TOT = B * C * H * W  # 98304 elements in the loss means
NG = 8                 # y-groups per core
GR = HS // NG          # 4 rows per group
GCOLS = GR * 132       # 528 shifted-row cols per group
GW = GCOLS + NG * 18   # + per-group masked weight blocks (g,dx,half)
LN67 = float(np.log(6.0 / 7.0))

_built = None
LAST_RESULTS = None


def _build_nc():
    import concourse.bacc as bacc
    import concourse.mybir as mybir
    from concourse import bass_isa
    from concourse import tile as tile_mod

    f32 = mybir.dt.float32
    f16 = mybir.dt.bfloat16
    i32 = mybir.dt.int32
    AF = mybir.ActivationFunctionType
    ALU = mybir.AluOpType
    AX = mybir.AxisListType
    ET = mybir.EngineType

    nc = bacc.Bacc()

    mpat_d = nc.dram_tensor("mpat", [NG * 10, GW], f16, kind="ExternalInput")
    win_d = nc.dram_tensor("win", [128, 8 * HS * C], f16, kind="ExternalInput")
    out_d = nc.dram_tensor("out", [1, 4], f32, kind="ExternalOutput")

    with tile_mod.TileContext(nc) as tc:
        with (
            tc.tile_pool(name="sb", bufs=1) as sb,
            tc.tile_pool(name="ps", bufs=1, space="PSUM") as ps,
        ):
            mpat = sb.tile([NG * 10, GW], f16)
            win = sb.tile([128, 8, HS, C], f16)

            # Pool carries the big win(+wz) DMA on the SWDGE queue; the
            # Pool engine is held for the whole transfer, which makes
            # its first consumer (m01) register its wait late for free.
            nc.gpsimd.dma_start(out=win[:].rearrange("p s h c -> p (s h c)"), in_=win_d[:])
            i_mdma = nc.sync.dma_start(out=mpat[:], in_=mpat_d[:])
            gate = nc.alloc_semaphore("pe_gate")
            i_ginc = nc.sync.sem_inc(gate, 1)
            tile_mod.add_dep_helper(i_ginc.ins, i_mdma.ins, reason="gate after mpat")

            # Explicit ACT table load (covers Exp + Ln); runs during the
            # DMAs and finishes at ~1483.
            tabs = bacc.get_activation_tables(nc.m.arch)
            set_id = list(tabs).index("natural_log_exp_and_others")
            atl = mybir.InstLoadActFuncSet(
                name=nc.get_next_instruction_name(), ins=[], outs=[],
                act_func_set_id=set_id,
            )
            i_atl = nc.scalar.add_instruction(atl)

            # DVE const tiles double as late-wait fillers: they keep DVE
            # busy past the win transfer-end (~690) so sq's wait
            # registers late.
            ceps = sb.tile([128, HS, C], f32)
            nc.vector.memset(ceps[:], EPS)
            c35 = sb.tile([128, HS, C], f32)
            i_c35 = nc.vector.memset(c35[:], 3.5)
            sq = sb.tile([128, 8, HS, C], f16)
            i_ms7 = nc.vector.memset(sq[:, 7:8], 0.0)
            tile_mod.add_dep_helper(i_ms7.ins, i_c35.ins, reason="DVE order")
            fill = sb.tile([128, HS, C], f16)
            i_f4 = nc.vector.memset(fill[:], 0.0)
            tile_mod.add_dep_helper(i_f4.ins, i_ms7.ins, reason="DVE order")

            # ---- conv: err[x, y, o] via accumulating matmuls ----
            # sigma channels first (unblocks the Pool clamp chain), then
            # the 2*p_mu channels. bias+temb folded into the ones-row of
            # the dx=0 weight block.
            # The contraction spans all 80 partitions from base 0 (PE
            # requires base 0/32/64); per-group selection happens via
            # zero-masked weight column blocks.
            i_gatew = nc.tensor.wait_ge(gate, 1)
            sig_ps = ps.tile([128, HS, C], f32)
            pm2_ps = ps.tile([128, HS, C], f32)
            for half in range(2):
                for y in range(HS):
                    g, yy = divmod(y, GR)
                    out_ps = sig_ps if half == 0 else pm2_ps
                    for dx in range(3):
                        wc = GCOLS + (g * 6 + dx * 2 + half) * 3
                        i_mm = nc.tensor.matmul(
                            out_ps[:, y, :],
                            mpat[0 : NG * 10,
                                 yy * 132 + dx : yy * 132 + dx + 128],
                            mpat[0 : NG * 10, wc : wc + 3],
                            start=(dx == 0),
                            stop=(dx == 2),
                        )
                        tile_mod.add_dep_helper(i_mm.ins, i_gatew.ins, reason="PE gate")

            # ---- Pool: musum tree, then the sampling/KL TT chain ----
            def pool_tt(name_tile, a, b, op):
                t = sb.tile([128, HS, C], f32, name=name_tile)
                ins = nc.gpsimd.tensor_tensor(t[:], a, b, op=op)
                return t, ins

            m01, i1 = pool_tt("m01", win[:, 0], win[:, 1], ALU.add)
            m23, i2 = pool_tt("m23", win[:, 2], win[:, 3], ALU.add)
            tile_mod.add_dep_helper(i2.ins, i1.ins, reason="pool order")
            m45, i3 = pool_tt("m45", win[:, 4], win[:, 5], ALU.add)
            tile_mod.add_dep_helper(i3.ins, i2.ins, reason="pool order")
            mA, i4 = pool_tt("mA", m01[:], m23[:], ALU.add)
            tile_mod.add_dep_helper(i4.ins, i3.ins, reason="pool order")
            mB, i5 = pool_tt("mB", m45[:], win[:, 6], ALU.add)
            tile_mod.add_dep_helper(i5.ins, i4.ins, reason="pool order")
            musum, i6 = pool_tt("musum", mA[:], mB[:], ALU.add)
            tile_mod.add_dep_helper(i6.ins, i5.ins, reason="pool order")
            bt2, i7 = pool_tt("bt2", musum[:], musum[:], ALU.mult)
            tile_mod.add_dep_helper(i7.ins, i6.ins, reason="pool order")
            sgs, i8 = pool_tt("sgs", sig_ps[:], ceps[:], ALU.max)
            tile_mod.add_dep_helper(i8.ins, i7.ins, reason="pool order")
            a1, i9 = pool_tt("a1", sgs[:], win[:, 7], ALU.add)
            tile_mod.add_dep_helper(i9.ins, i8.ins, reason="pool order")
            t2, i10 = pool_tt("t2", sgs[:], a1[:], ALU.mult)
            tile_mod.add_dep_helper(i10.ins, i9.ins, reason="pool order")
            ein, i11 = pool_tt("ein", t2[:], pm2_ps[:], ALU.add)
            tile_mod.add_dep_helper(i11.ins, i10.ins, reason="pool order")
            sg2, i12 = pool_tt("sg2", sgs[:], sgs[:], ALU.mult)
            tile_mod.add_dep_helper(i12.ins, i11.ins, reason="pool order")
            dm, i13 = pool_tt("dm", pm2_ps[:], c35[:], ALU.mult)
            tile_mod.add_dep_helper(i13.ins, i12.ins, reason="pool order")
            dmu, i14 = pool_tt("dmu", dm[:], musum[:], ALU.subtract)
            tile_mod.add_dep_helper(i14.ins, i13.ins, reason="pool order")
            d7sq, i15 = pool_tt("d7sq", dmu[:], dmu[:], ALU.mult)
            tile_mod.add_dep_helper(i15.ins, i14.ins, reason="pool order")

            # ---- DVE: squares tree -> ssq -> v6 -> u,c2 divides ----
            nc.vector.tensor_tensor(sq[:, 0:7], win[:, 0:7], win[:, 0:7], op=ALU.mult)
            u4 = sb.tile([128, 4, HS, C], f16)
            nc.vector.tensor_tensor(u4[:], sq[:, 0:4], sq[:, 4:8], op=ALU.add)
            u2 = sb.tile([128, 2, HS, C], f16)
            nc.vector.tensor_tensor(u2[:], u4[:, 0:2], u4[:, 2:4], op=ALU.add)
            ssq = sb.tile([128, HS, C], f32)
            nc.vector.tensor_tensor(ssq[:], u2[:, 0], u2[:, 1], op=ALU.add)
            v6 = sb.tile([128, HS, C], f32)
            nc.vector.scalar_tensor_tensor(
                v6[:], ssq[:], 7.0, bt2[:], op0=ALU.mult, op1=ALU.subtract
            )
            cols = sb.tile([128, 4], f32)
            u = sb.tile([128, HS, C], f32)
            i_u = nc.vector.scalar_tensor_tensor(
                u[:], sg2[:], 49.0, v6[:], op0=ALU.mult, op1=ALU.divide,
                accum_out=cols[:, 1:2],
            )
            c2 = sb.tile([128, HS, C], f32)
            i_c2 = nc.vector.scalar_tensor_tensor(
                c2[:], d7sq[:], 1.0, v6[:], op0=ALU.mult, op1=ALU.divide,
                accum_out=cols[:, 2:3],
            )
            tile_mod.add_dep_helper(i_c2.ins, i_u.ins, reason="DVE u first")

            # ---- ACT: Exp then Ln(u), both after the ATL ----
            e = sb.tile([128, HS, C], f32)
            i_e = nc.scalar.activation(e[:], ein[:], AF.Exp, scale=0.5)
            tile_mod.add_dep_helper(i_e.ins, i_atl.ins, reason="ACT order atl<e")
            lnu = sb.tile([128, HS, C], f32)
            i_lnu = nc.scalar.activation(
                lnu[:], u[:], AF.Ln, accum_out=cols[:, 3:4]
            )
            tile_mod.add_dep_helper(i_lnu.ins, i_e.ins, reason="ACT order e<lnu")

            # ---- sampling tail ----
            xt, i16 = pool_tt("xt", e[:], win[:, 2], ALU.mult)
            tile_mod.add_dep_helper(i16.ins, i15.ins, reason="pool order")
            d, i17 = pool_tt("d", xt[:], win[:, 3], ALU.subtract)
            tile_mod.add_dep_helper(i17.ins, i16.ins, reason="pool order")
            i_dred = nc.vector.tensor_reduce(
                cols[:, 0:1], d[:], axis=AX.XY, op=ALU.add,
                apply_absolute_value=True,
            )
            tile_mod.add_dep_helper(i_dred.ins, i_c2.ins, reason="DVE order")

            # ---- final: one partition all-reduce + register stores ----
            red = sb.tile([128, 4], f32)
            i_par = nc.gpsimd.partition_all_reduce(
                red[:], cols[:], 128, bass_isa.ReduceOp.add
            )
            tile_mod.add_dep_helper(i_par.ins, i17.ins, reason="pool order")
            regs = [nc.alloc_register(ET.Pool, f"acc{i}") for i in range(4)]
            nc.gpsimd.reg_load(regs, red[0:1, 0:4].bitcast(i32))
            for i in range(4):
                nc.gpsimd.store(out=out_d[0:1, i : i + 1].bitcast(i32), in_=regs[i])

    # The explicit ATL above covers Exp and Ln; suppress the
    # compile-time auto-inserter (it would add a second 1283ns load).
    nc.insert_act_table_loads = lambda: None
    nc.compile()
    return nc


def _prep_inputs(x, z, Wm, bm, temb_w, t):
    """Build the 8 per-core input dicts (pure numpy, host side)."""
    x = np.ascontiguousarray(np.asarray(x, dtype=np.float32))
    z = np.asarray(z, dtype=np.float32)
    Wm = np.asarray(Wm, dtype=np.float32)
    bm = np.asarray(bm, dtype=np.float32)
    temb_w = np.asarray(temb_w, dtype=np.float32)
    t = np.asarray(t)

    in_maps = []
    for b in range(B):
        ti = int(t[b])
        st = min(max(ti - K, 0), T - (2 * K + 1))
        window = x[b, st : st + 2 * K + 1]          # [7,3,128,128]
        xin = window[K - 1]                          # [3,128,128]
        bias = bm + temb_w * (np.float32(ti) / np.float32(T))
        sqt2 = np.float32(2.0 * np.sqrt(np.float64(ti)))

        # weight columns, shared across quarters: [dx, p(c,dy)+ones, o]
        wcols = np.zeros((3, 10, 6), np.float32)
        for dx in range(3):
            for c in range(C):
                for dy in range(3):
                    p = c * 3 + dy
                    wcols[dx, p, 0:3] = Wm[3:6, c, dy, dx]
                    wcols[dx, p, 3:6] = 2.0 * Wm[0:3, c, dy, dx]
        wcols[0, 9, 0:3] = bias[3:6]
        wcols[0, 9, 3:6] = 2.0 * bias[0:3]

        for q in range(4):
            r0 = q * HS
            # win: [w, s, h, c] with wz as slab 7
            wina = np.empty((128, 8, HS, C), np.float32)
            wina[:, 0:7] = window[:, :, r0 : r0 + HS, :].transpose(3, 0, 2, 1)
            wina[:, 7] = (sqt2 * z[b, :, r0 : r0 + HS, :]).transpose(2, 1, 0)
            wina = wina.reshape(128, 8 * HS * C).astype(NPBF16)
            # mpat: 8 groups of 4 padded shifted rows + ones + weights
            XP = np.zeros((C, HS + 2, 132), np.float32)
            XP[:, 1 : HS + 1, 1:129] = xin[:, r0 : r0 + HS, :]
            if r0 > 0:
                XP[:, 0, 1:129] = xin[:, r0 - 1, :]
            if r0 + HS < H:
                XP[:, HS + 1, 1:129] = xin[:, r0 + HS, :]
            mpat = np.zeros((NG * 10, GW), np.float32)
            for g in range(NG):
                y0 = g * GR
                for c in range(C):
                    for dy in range(3):
                        mpat[10 * g + c * 3 + dy, :GCOLS] = (
                            XP[c, y0 + dy : y0 + dy + GR, :].reshape(-1)
                        )
                mpat[10 * g + 9, :GCOLS] = 1.0
                for dx in range(3):
                    for half in range(2):
                        wc = GCOLS + (g * 6 + dx * 2 + half) * 3
                        mpat[10 * g : 10 * g + 10, wc : wc + 3] = (
                            wcols[dx, :, 3 * half : 3 * half + 3]
                        )
            in_maps.append({
                "mpat": mpat.astype(NPBF16),
                "win": wina,
            })
    return in_maps


def _combine(results):
    outs = np.stack([np.asarray(r["out"], dtype=np.float64) for r in results])
    s = outs.sum(axis=0)[0]  # [4]: sum|d|, sum u, sum d7sq/v6, sum ln u
    l1 = s[0] / N_TOT
    kl_sum = 0.5 * ((6.0 / 7.0) * (s[1] + s[2]) - s[3] - N_TOT * LN67 - N_TOT)
    return np.float32(l1 + kl_sum / N_TOT)


def kernel(x, z, Wm, bm, temb_w, t):
    global _built, LAST_RESULTS
    from concourse.bass_utils import run_bass_kernel_spmd

    if _built is None:
        _built = _build_nc()
    nc = _built

    in_maps = _prep_inputs(x, z, Wm, bm, temb_w, t)
    trace = bool(os.environ.get("BASS_TRACE"))
    res = run_bass_kernel_spmd(nc, in_maps, core_ids=list(range(N_CORES)), trace=trace)
    LAST_RESULTS = res
    return _combine(res.results)


# revision 13
# speedup vs baseline: 1.7788x; 1.0198x over previous
"""Trainium2 Bass kernel for the GBM sampling-loss problem (v3).

Contract: kernel(**inputs) takes the FULL unsharded inputs
  x[2,500,3,128,128] z[2,3,128,128] Wm[6,3,3,3] bm[6] temb_w[6] t[2]
and returns the scalar loss (np.float32, shape ()).

v3 redesign vs the 3788ns v2, driven by trace analysis:
  - conv via an 8-group x 10-partition layout [(g,c,dy)+ones, 4*132
    shifted row cols + weights]: 1092B/partition hits the ~500ns DMA
    floor (DMA cost is per-partition bytes, not total), vs 500ns for
    the v2 im2col pair. 2x96 tiny matmuls with 3-way PSUM accumulation
    over dx; the sigma channels run as a separate first pass so the
    Pool clamp chain starts ~250ns before the pm2 channels finish.
  - ACT carries only Exp and ONE Ln: ln(var_ratio) = ln(49*sig^2/v6)
    + const, so u = 49*sig^2/v6 (one DVE STT divide) replaces the
    v2 lnS/lnG pair; the Relu clamp moves to Pool (TT max vs an EPS
    tile, reading sigma straight from PSUM) and the p_mu PSUM copy is
    gone (Pool reads the 2*p_mu channels from PSUM directly).
  - KL t1 term via (3.5*(2pm) - musum)^2 / v6 with STT accumulate.
  - wz rides the win DMA as slab 7 (one [128, 8, 32, 3] SWDGE DMA).
  - Output: 4 per-partition column sums -> one partition_all_reduce ->
    Pool register stores (no output DMA).
  - Every cross-engine consumer of a DMA is scheduled to issue after
    the transfer-end (filler memsets / engine busyness), so no consumer
    pays the +1717ns DMA-semaphore path.

Engine layout:
  SP   : mpat DMA, then the PE gate sem
  Pool : win DMA (SWDGE), musum tree, bt2, clamp, a1/t2/ein chain,
         sg2/dm/dmu/d7sq, xt/d, partition_all_reduce + register stores
  DVE  : const memsets (double as late-wait fillers), sq tree, v6,
         u/c2 STT divides (with accum cols), |d| reduce
  ACT  : [ATL], Exp, Ln(u) with accum col
  PE   : 2x96 tiny conv matmuls (dx-accumulated), gated on the SP sem
"""

import os
import sys

sys.path.insert(0, "/opt/trn_rl_repo")

import numpy as np

try:
    import ml_dtypes
    NPBF16 = np.dtype(ml_dtypes.bfloat16)
except ImportError:  # pragma: no cover
    import jax.numpy as jnp
    NPBF16 = np.dtype(jnp.bfloat16)

K = 3
T = 500
C = 3
B = 2
H = 128
W = 128
EPS = 1e-7
N_CORES = 8
HS = H // 4            # 32 rows per core
N_TOT = B * C * H * W  # 98304 elements in the loss means
NG = 8                 # y-groups per core
GR = HS // NG          # 4 rows per group
GCOLS = GR * 132       # 528 shifted-row cols per group
GW = GCOLS + NG * 18   # + per-group masked weight blocks (g,dx,half)
LN67 = float(np.log(6.0 / 7.0))

_built = None
LAST_RESULTS = None


def _build_nc():
    import concourse.bacc as bacc
    import concourse.mybir as mybir
    from concourse import bass_isa
    from concourse import tile as tile_mod

    f32 = mybir.dt.float32
    f16 = mybir.dt.bfloat16
    i32 = mybir.dt.int32
    AF = mybir.ActivationFunctionType
    ALU = mybir.AluOpType
    AX = mybir.AxisListType
    ET = mybir.EngineType

    nc = bacc.Bacc()

    mpat_d = nc.dram_tensor("mpat", [NG * 10, GW], f16, kind="ExternalInput")
    win_d = nc.dram_tensor("win", [128, 7 * HS * C], f16, kind="ExternalInput")
    wz_d = nc.dram_tensor("wz", [128, HS * C], f16, kind="ExternalInput")
    out_d = nc.dram_tensor("out", [1, 4], f32, kind="ExternalOutput")

    with tile_mod.TileContext(nc) as tc:
        with (
            tc.tile_pool(name="sb", bufs=1) as sb,
            tc.tile_pool(name="ps", bufs=1, space="PSUM") as ps,
        ):
            mpat = sb.tile([NG * 10, GW], f16)
            win = sb.tile([128, 7, HS, C], f16)
            wz = sb.tile([128, HS, C], f16)

            # Pool carries the big win(+wz) DMA on the SWDGE queue; the
            # Pool engine is held for the whole transfer, which makes
            # its first consumer (m01) register its wait late for free.
            nc.gpsimd.dma_start(out=win[:].rearrange("p s h c -> p (s h c)"), in_=win_d[:])
            i_mdma = nc.sync.dma_start(out=mpat[:], in_=mpat_d[:])
            gate = nc.alloc_semaphore("pe_gate")
            i_ginc = nc.sync.sem_inc(gate, 1)
            tile_mod.add_dep_helper(i_ginc.ins, i_mdma.ins, reason="gate after mpat")
            i_wzdma = nc.sync.dma_start(out=wz[:].rearrange("p h c -> p (h c)"), in_=wz_d[:])
            tile_mod.add_dep_helper(i_wzdma.ins, i_mdma.ins, reason="SP dma order")

            # Explicit ACT table load (covers Exp + Ln); runs during the
            # DMAs and finishes at ~1483.
            tabs = bacc.get_activation_tables(nc.m.arch)
            set_id = list(tabs).index("natural_log_exp_and_others")
            atl = mybir.InstLoadActFuncSet(
                name=nc.get_next_instruction_name(), ins=[], outs=[],
                act_func_set_id=set_id,
            )
            i_atl = nc.scalar.add_instruction(atl)

            # DVE const tiles double as late-wait fillers: they keep DVE
            # busy past the win transfer-end (~690) so sq's wait
            # registers late.
            ceps = sb.tile([128, HS, C], f32)
            nc.vector.memset(ceps[:], EPS)
            c35 = sb.tile([128, HS, C], f32)
            i_c35 = nc.vector.memset(c35[:], 3.5)
            sq = sb.tile([128, 8, HS, C], f16)
            i_ms7 = nc.vector.memset(sq[:, 7:8], 0.0)
            tile_mod.add_dep_helper(i_ms7.ins, i_c35.ins, reason="DVE order")

            # ---- conv: err[x, y, o] via accumulating matmuls ----
            # sigma channels first (unblocks the Pool clamp chain), then
            # the 2*p_mu channels. bias+temb folded into the ones-row of
            # the dx=0 weight block.
            # The contraction spans all 80 partitions from base 0 (PE
            # requires base 0/32/64); per-group selection happens via
            # zero-masked weight column blocks.
            i_gatew = nc.tensor.wait_ge(gate, 1)
            sig_ps = ps.tile([128, HS, C], f32)
            pm2_ps = ps.tile([128, HS, C], f32)
            for half in range(2):
                for y in range(HS):
                    g, yy = divmod(y, GR)
                    out_ps = sig_ps if half == 0 else pm2_ps
                    for dx in range(3):
                        wc = GCOLS + (g * 6 + dx * 2 + half) * 3
                        i_mm = nc.tensor.matmul(
                            out_ps[:, y, :],
                            mpat[0 : NG * 10,
                                 yy * 132 + dx : yy * 132 + dx + 128],
                            mpat[0 : NG * 10, wc : wc + 3],
                            start=(dx == 0),
                            stop=(dx == 2),
                        )
                        tile_mod.add_dep_helper(i_mm.ins, i_gatew.ins, reason="PE gate")

            # ---- Pool: musum tree, then the sampling/KL TT chain ----
            def pool_tt(name_tile, a, b, op):
                t = sb.tile([128, HS, C], f32, name=name_tile)
                ins = nc.gpsimd.tensor_tensor(t[:], a, b, op=op)
                return t, ins

            m01, i1 = pool_tt("m01", win[:, 0], win[:, 1], ALU.add)
            m23, i2 = pool_tt("m23", win[:, 2], win[:, 3], ALU.add)
            tile_mod.add_dep_helper(i2.ins, i1.ins, reason="pool order")
            m45, i3 = pool_tt("m45", win[:, 4], win[:, 5], ALU.add)
            tile_mod.add_dep_helper(i3.ins, i2.ins, reason="pool order")
            mA, i4 = pool_tt("mA", m01[:], m23[:], ALU.add)
            tile_mod.add_dep_helper(i4.ins, i3.ins, reason="pool order")
            mB, i5 = pool_tt("mB", m45[:], win[:, 6], ALU.add)
            tile_mod.add_dep_helper(i5.ins, i4.ins, reason="pool order")
            musum, i6 = pool_tt("musum", mA[:], mB[:], ALU.add)
            tile_mod.add_dep_helper(i6.ins, i5.ins, reason="pool order")
            bt2, i7 = pool_tt("bt2", musum[:], musum[:], ALU.mult)
            tile_mod.add_dep_helper(i7.ins, i6.ins, reason="pool order")
            sgs, i8 = pool_tt("sgs", sig_ps[:], ceps[:], ALU.max)
            tile_mod.add_dep_helper(i8.ins, i7.ins, reason="pool order")
            a1, i9 = pool_tt("a1", sgs[:], wz[:], ALU.add)
            tile_mod.add_dep_helper(i9.ins, i8.ins, reason="pool order")
            t2, i10 = pool_tt("t2", sgs[:], a1[:], ALU.mult)
            tile_mod.add_dep_helper(i10.ins, i9.ins, reason="pool order")
            ein, i11 = pool_tt("ein", t2[:], pm2_ps[:], ALU.add)
            tile_mod.add_dep_helper(i11.ins, i10.ins, reason="pool order")
            sg2, i12 = pool_tt("sg2", sgs[:], sgs[:], ALU.mult)
            tile_mod.add_dep_helper(i12.ins, i11.ins, reason="pool order")
            dm, i13 = pool_tt("dm", pm2_ps[:], c35[:], ALU.mult)
            tile_mod.add_dep_helper(i13.ins, i12.ins, reason="pool order")
            dmu, i14 = pool_tt("dmu", dm[:], musum[:], ALU.subtract)
            tile_mod.add_dep_helper(i14.ins, i13.ins, reason="pool order")
            d7sq, i15 = pool_tt("d7sq", dmu[:], dmu[:], ALU.mult)
            tile_mod.add_dep_helper(i15.ins, i14.ins, reason="pool order")

            # ---- DVE: squares tree -> ssq -> v6 -> u,c2 divides ----
            nc.vector.tensor_tensor(sq[:, 0:7], win[:, 0:7], win[:, 0:7], op=ALU.mult)
            u4 = sb.tile([128, 4, HS, C], f16)
            nc.vector.tensor_tensor(u4[:], sq[:, 0:4], sq[:, 4:8], op=ALU.add)
            u2 = sb.tile([128, 2, HS, C], f16)
            nc.vector.tensor_tensor(u2[:], u4[:, 0:2], u4[:, 2:4], op=ALU.add)
            ssq = sb.tile([128, HS, C], f32)
            nc.vector.tensor_tensor(ssq[:], u2[:, 0], u2[:, 1], op=ALU.add)
            v6 = sb.tile([128, HS, C], f32)
            nc.vector.scalar_tensor_tensor(
                v6[:], ssq[:], 7.0, bt2[:], op0=ALU.mult, op1=ALU.subtract
            )
            cols = sb.tile([128, 3], f32)
            u = sb.tile([128, HS, C], f32)
            i_u = nc.vector.scalar_tensor_tensor(
                u[:], sg2[:], 49.0, v6[:], op0=ALU.mult, op1=ALU.divide,
                accum_out=cols[:, 1:2],
            )
            c2 = sb.tile([128, HS, C], f32)
            i_c2 = nc.vector.scalar_tensor_tensor(
                c2[:], d7sq[:], 1.0, v6[:], op0=ALU.mult, op1=ALU.divide,
                accum_out=cols[:, 2:3],
            )
            tile_mod.add_dep_helper(i_c2.ins, i_u.ins, reason="DVE u first")

            # ---- ACT: Exp then Ln(u), both after the ATL ----
            e = sb.tile([128, HS, C], f32)
            i_e = nc.scalar.activation(e[:], ein[:], AF.Exp, scale=0.5)
            tile_mod.add_dep_helper(i_e.ins, i_atl.ins, reason="ACT order atl<e")
            lnu = sb.tile([128, HS, C], f32)
            i_lnu = nc.scalar.activation(lnu[:], u[:], AF.Ln)
            tile_mod.add_dep_helper(i_lnu.ins, i_e.ins, reason="ACT order e<lnu")

            # ---- sampling tail ----
            xt, i16 = pool_tt("xt", e[:], win[:, 2], ALU.mult)
            tile_mod.add_dep_helper(i16.ins, i15.ins, reason="pool order")
            d, i17 = pool_tt("d", xt[:], win[:, 3], ALU.subtract)
            tile_mod.add_dep_helper(i17.ins, i16.ins, reason="pool order")
            i_dred = nc.vector.tensor_reduce(
                cols[:, 0:1], d[:], axis=AX.XY, op=ALU.add,
                apply_absolute_value=True,
            )
            tile_mod.add_dep_helper(i_dred.ins, i_c2.ins, reason="DVE order")

            # ---- final: partition all-reduce + lnu XYZWC + reg stores ----
            red = sb.tile([128, 4], f32)
            i_par = nc.gpsimd.partition_all_reduce(
                red[:, 0:3], cols[:], 128, bass_isa.ReduceOp.add
            )
            tile_mod.add_dep_helper(i_par.ins, i17.ins, reason="pool order")
            i_lred = nc.gpsimd.tensor_reduce(
                red[0:1, 3:4], lnu[:], axis=AX.XYZWC, op=ALU.add
            )
            tile_mod.add_dep_helper(i_lred.ins, i_par.ins, reason="pool order")
            regs = [nc.alloc_register(ET.Pool, f"acc{i}") for i in range(4)]
            nc.gpsimd.reg_load(regs, red[0:1, 0:4].bitcast(i32))
            for i in range(4):
                nc.gpsimd.store(out=out_d[0:1, i : i + 1].bitcast(i32), in_=regs[i])

    # The explicit ATL above covers Exp and Ln; suppress the
    # compile-time auto-inserter (it would add a second 1283ns load).
    nc.insert_act_table_loads = lambda: None
    nc.compile()
    return nc


def _prep_inputs(x, z, Wm, bm, temb_w, t):
    """Build the 8 per-core input dicts (pure numpy, host side)."""
    x = np.ascontiguousarray(np.asarray(x, dtype=np.float32))
    z = np.asarray(z, dtype=np.float32)
    Wm = np.asarray(Wm, dtype=np.float32)
    bm = np.asarray(bm, dtype=np.float32)
    temb_w = np.asarray(temb_w, dtype=np.float32)
    t = np.asarray(t)

    in_maps = []
    for b in range(B):
        ti = int(t[b])
        st = min(max(ti - K, 0), T - (2 * K + 1))
        window = x[b, st : st + 2 * K + 1]          # [7,3,128,128]
        xin = window[K - 1]                          # [3,128,128]
        bias = bm + temb_w * (np.float32(ti) / np.float32(T))
        sqt2 = np.float32(2.0 * np.sqrt(np.float64(ti)))

        # weight columns, shared across quarters: [dx, p(c,dy)+ones, o]
        wcols = np.zeros((3, 10, 6), np.float32)
        for dx in range(3):
            for c in range(C):
                for dy in range(3):
                    p = c * 3 + dy
                    wcols[dx, p, 0:3] = Wm[3:6, c, dy, dx]
                    wcols[dx, p, 3:6] = 2.0 * Wm[0:3, c, dy, dx]
        wcols[0, 9, 0:3] = bias[3:6]
        wcols[0, 9, 3:6] = 2.0 * bias[0:3]

        for q in range(4):
            r0 = q * HS
            # win: [w, s, h, c]; wz separate
            wina = window[:, :, r0 : r0 + HS, :].transpose(3, 0, 2, 1)
            wina = np.ascontiguousarray(wina).reshape(128, 7 * HS * C).astype(NPBF16)
            wzv = (sqt2 * z[b, :, r0 : r0 + HS, :]).transpose(2, 1, 0)
            wzv = np.ascontiguousarray(wzv).reshape(128, HS * C).astype(NPBF16)
            # mpat: 8 groups of 4 padded shifted rows + ones + weights
            XP = np.zeros((C, HS + 2, 132), np.float32)
            XP[:, 1 : HS + 1, 1:129] = xin[:, r0 : r0 + HS, :]
            if r0 > 0:
                XP[:, 0, 1:129] = xin[:, r0 - 1, :]
            if r0 + HS < H:
                XP[:, HS + 1, 1:129] = xin[:, r0 + HS, :]
            mpat = np.zeros((NG * 10, GW), np.float32)
            for g in range(NG):
                y0 = g * GR
                for c in range(C):
                    for dy in range(3):
                        mpat[10 * g + c * 3 + dy, :GCOLS] = (
                            XP[c, y0 + dy : y0 + dy + GR, :].reshape(-1)
                        )
                mpat[10 * g + 9, :GCOLS] = 1.0
                for dx in range(3):
                    for half in range(2):
                        wc = GCOLS + (g * 6 + dx * 2 + half) * 3
                        mpat[10 * g : 10 * g + 10, wc : wc + 3] = (
                            wcols[dx, :, 3 * half : 3 * half + 3]
                        )
            in_maps.append({
                "mpat": mpat.astype(NPBF16),
                "win": wina,
                "wz": wzv,
            })
    return in_maps


def _combine(results):
    outs = np.stack([np.asarray(r["out"], dtype=np.float64) for r in results])
    s = outs.sum(axis=0)[0]  # [4]: sum|d|, sum u, sum d7sq/v6, sum ln u
    l1 = s[0] / N_TOT
    kl_sum = 0.5 * ((6.0 / 7.0) * (s[1] + s[2]) - s[3] - N_TOT * LN67 - N_TOT)
    return np.float32(l1 + kl_sum / N_TOT)


def kernel(x, z, Wm, bm, temb_w, t):
    global _built, LAST_RESULTS
    from concourse.bass_utils import run_bass_kernel_spmd

    if _built is None:
        _built = _build_nc()
    nc = _built

    in_maps = _prep_inputs(x, z, Wm, bm, temb_w, t)
    trace = bool(os.environ.get("BASS_TRACE"))
    res = run_bass_kernel_spmd(nc, in_maps, core_ids=list(range(N_CORES)), trace=trace)
    LAST_RESULTS = res
    return _combine(res.results)
